# revision 13
# baseline (speedup 1.0000x reference)
"""GAT (3-layer, 8-head) forward on 8 Trainium2 NeuronCores.

Strategy:
  - Shard nodes (and their incoming edges) across 8 cores: core c owns dst
    nodes [c*12500, (c+1)*12500).
  - Per layer: node features+attention logits live in a replicated DRAM
    "table" (built via AllGather).  Each core gathers table rows for its
    edges' src nodes (dma_gather, int16 idx => 4 table "supers"), computes
    edge softmax weights p = exp(leakyrelu(al_src[s] + al_dst[d])) with NO
    max-shift (logit ranges are small; softmax is shift invariant), and
    segment-sums p*h via one-hot matmuls on the TensorEngine into per-
    128-dst-window PSUM accumulators.  Window flush divides by the summed
    p (denominator), applies bias/ELU/BatchNorm (BN folded into the next
    layer's weights) and computes the next layer's table rows.
"""
import os
import sys
import time
import numpy as np

sys.path.insert(0, '/opt/trn_rl_repo')

import concourse.bass as bass
import concourse.bacc as bacc
import concourse.mybir as mybir
import concourse.tile as tile
from concourse.bass_utils import run_bass_kernel_spmd
import ml_dtypes

# ---------------- problem constants (hardcoded per contract) ----------------
F_IN = 500
HID = 16
HEADS = 8
NCLS = 10
NEG = 0.2
BN_EPS = 1e-5
NC = 8                      # cores


def _config(n=100000, e=1600000):
    g = globals()
    g['N'] = n
    g['E'] = e
    g['NSH'] = n // NC
    g['NWIN'] = (g['NSH'] + 127) // 128
    g['NSHP'] = g['NWIN'] * 128
    g['SUPR'] = (NC * g['NSHP']) // NSUP
    assert g['SUPR'] <= 32767


NSUP = 4                    # table row supers (int16 gather idx limit)
WG = 2                      # windows per group
GCH = 512                   # max idxs per dma_gather
_config()
F1 = HEADS * HID            # 128
TW = 256                    # bf16 table row width (512B): [h(128)|al_src f32(8=16slots)|pad]
TW3 = 64                    # f32 layer-3 table row width (256B)
FP32 = mybir.dt.float32
BF16 = mybir.dt.bfloat16
I16 = mybir.dt.int16

_CACHE = {}
LAST = {}


def _wrap16(a):
    # idx i -> [i%16, i//16]; replicated over the 8 16-partition groups
    n = a.shape[0]
    w = a.reshape(n // 16, 16).T
    return np.tile(w, (8, 1))


def _stripe(a):
    # edge i -> [i%128, i//128]
    n = a.shape[0]
    return a.reshape(n // 128, 128, *a.shape[1:]).swapaxes(0, 1)


def _preprocess(edge_index):
    """Partition + order edges; build per-core gather/dst arrays and the
    (SPMD-uniform) program structure."""
    src = np.concatenate([edge_index[0], np.arange(N, dtype=np.int64)])
    dst = np.concatenate([edge_index[1], np.arange(N, dtype=np.int64)])
    core = dst // NSH
    dloc = dst % NSH
    win = dloc // 128
    wloc = dloc % 128
    trow = (src // NSH) * NSHP + (src % NSH)   # table row (padded shards)
    sup = trow // SUPR
    lidx = trow % SUPR

    # group edges by (core, window, super)
    order = np.lexsort((lidx, sup, win, core))
    core_s, win_s, sup_s = core[order], win[order], sup[order]
    wloc_s, lidx_s = wloc[order], lidx[order]

    # counts[c, w, s]
    key = (core_s * NWIN + win_s) * NSUP + sup_s
    counts = np.bincount(key, minlength=NC * NWIN * NSUP).reshape(NC, NWIN, NSUP)
    L = counts.max(axis=0)                      # [NWIN, NSUP] uniform sub-seg lens
    starts = np.concatenate([[0], np.cumsum(counts.ravel())]).astype(np.int64)

    # groups of WG windows
    groups = [list(range(g, min(g + WG, NWIN))) for g in range(0, NWIN, WG)]
    # P[g][s]: group-super segment length (128-aligned)
    P = np.zeros((len(groups), NSUP), np.int64)
    for gi, ws in enumerate(groups):
        for s in range(NSUP):
            tot = int(L[ws, s].sum())
            P[gi, s] = (tot + 127) // 128 * 128

    EPAD = int(P.sum())

    # per-core flat arrays in processing order
    idx_flat = np.zeros((NC, EPAD), np.int16)
    dst_flat = np.full((NC, EPAD), -1.0, np.float32)
    # per group: cols[c] = list of (w_rel, p_lo, p_hi, start, stop) entries;
    # full-column PE matmuls, masked variants for window-crossing columns
    pieces = []
    for gi, ws in enumerate(groups):
        ncols = int(P[gi].sum()) // 128
        colent = [[] for _ in range(ncols)]
        colbase = 0
        first_seen = set()
        last_ent = {}
        for s in range(NSUP):
            q = 0
            for wi, w in enumerate(ws):
                ln = int(L[w, s])
                for c in range(NC):
                    k = (c * NWIN + w) * NSUP + s
                    cnt = int(counts[c, w, s])
                    a = starts[k]
                    off = int(P[:gi].sum() + P[gi, :s].sum()) + q
                    idx_flat[c, off:off + cnt] = lidx_s[a:a + cnt]
                    dst_flat[c, off:off + cnt] = wloc_s[a:a + cnt]
                rem, qq = ln, q
                while rem > 0:
                    K = min(128 - qq % 128, rem)
                    st = w not in first_seen
                    first_seen.add(w)
                    ent = [wi, qq % 128, qq % 128 + K, st, False]
                    colent[colbase + qq // 128].append(ent)
                    last_ent[w] = ent
                    qq += K
                    rem -= K
                q += ln
            colbase += int(P[gi, s]) // 128
        for w in ws:
            last_ent[w][4] = True
        for ents in colent:
            assert len(ents) <= 2, "column spans >2 windows"
        pieces.append(colent)

    # permuted edge order (per core) for assembling idx arrays done above;
    # build wrapped/striped host tensors
    idx_w = np.stack([_wrap16(idx_flat[c]) for c in range(NC)])        # [NC,128,EPAD/16]
    dst_st = np.stack([_stripe(dst_flat[c]) for c in range(NC)])       # [NC,128,EPAD/128]
    dst_fl = dst_flat.reshape(NC, 1, EPAD)

    return dict(groups=groups, P=P, EPAD=EPAD, pieces=pieces,
                idx_w=idx_w.astype(np.int16), dst_st=np.ascontiguousarray(dst_st),
                dst_fl=np.ascontiguousarray(dst_fl).astype(ml_dtypes.bfloat16))


def _block_diag_att(a):
    # a: [H, C] -> A [H*C, H] with A[h*C+c, h] = a[h, c]
    Hh, C = a.shape
    A = np.zeros((Hh * C, Hh), np.float32)
    for h in range(Hh):
        A[h * C:(h + 1) * C, h] = a[h]
    return A


def _build(struct):
    groups, P, EPAD, pieces = struct['groups'], struct['P'], struct['EPAD'], struct['pieces']
    NGRP = len(groups)

    nc = bacc.Bacc("TRN2", target_bir_lowering=False, debug=False, num_devices=NC)

    # ---- parameters ----
    xT = nc.declare_dram_parameter("xT", [4, 128, NSHP], FP32, isOutput=False)
    W1e = nc.declare_dram_parameter("W1e", [4, 128, 144], FP32, isOutput=False)
    W2e = nc.declare_dram_parameter("W2e", [128, 144], FP32, isOutput=False)
    W3e = nc.declare_dram_parameter("W3e", [128, 12], FP32, isOutput=False)
    c2r = nc.declare_dram_parameter("c2r", [1, 144], FP32, isOutput=False)
    c3r = nc.declare_dram_parameter("c3r", [1, 12], FP32, isOutput=False)
    b1r = nc.declare_dram_parameter("b1r", [128, 128], FP32, isOutput=False)
    b2r = nc.declare_dram_parameter("b2r", [128, 128], FP32, isOutput=False)
    b3r = nc.declare_dram_parameter("b3r", [128, 10], FP32, isOutput=False)
    idxT = nc.declare_dram_parameter("idx", [128, EPAD // 16], I16, isOutput=False)
    dstS = nc.declare_dram_parameter("dsts", [128, EPAD // 128], FP32, isOutput=False)
    dstF = nc.declare_dram_parameter("dstf", [1, EPAD], BF16, isOutput=False)
    out_ext = nc.declare_dram_parameter("out", [NSH, 10], FP32, isOutput=True)

    # ---- internal DRAM ----
    table = nc.dram_tensor("table", [NC * NSHP, TW], BF16, addr_space="Shared")
    table3 = nc.dram_tensor("table3", [NC * NSHP, TW3], FP32, addr_space="Shared")
    agin = nc.dram_tensor("agin", [NSHP, TW], BF16)
    agin3 = nc.dram_tensor("agin3", [NSHP, TW3], FP32)

    def ewidth(layer):   # rhs width: msg | p
        return 136 if layer < 3 else 11

    with tile.TileContext(nc, num_cores=NC) as tc, \
         tc.tile_pool(name="consts", bufs=1) as cpool:
        # constants
        iota_col = cpool.tile([128, 1], FP32)
        nc.gpsimd.iota(iota_col[:], [[0, 1]], base=0, channel_multiplier=1,
                       allow_small_or_imprecise_dtypes=True)
        iota_row = cpool.tile([128, 128], FP32)
        nc.gpsimd.iota(iota_row[:], [[1, 128]], base=0, channel_multiplier=0,
                       allow_small_or_imprecise_dtypes=True)
        ident = cpool.tile([128, 128], FP32)
        nc.vector.tensor_scalar(ident[:], iota_row[:], iota_col[:], None,
                                mybir.AluOpType.is_equal)
        ones1 = cpool.tile([1, 128], FP32)
        nc.gpsimd.memset(ones1[:], 1.0)
        w2t = cpool.tile([128, 144], FP32)
        nc.sync.dma_start(w2t[:], W2e[:])
        w3t = cpool.tile([128, 12], FP32)
        nc.sync.dma_start(w3t[:], W3e[:])
        c2t = cpool.tile([1, 144], FP32)
        nc.sync.dma_start(c2t[:], c2r[:])
        c3t = cpool.tile([1, 12], FP32)
        nc.sync.dma_start(c3t[:], c3r[:])
        b1t = cpool.tile([128, 128], FP32)
        nc.sync.dma_start(b1t[:], b1r[:])
        b2t = cpool.tile([128, 128], FP32)
        nc.sync.dma_start(b2t[:], b2r[:])
        b3t = cpool.tile([128, 10], FP32)
        nc.sync.dma_start(b3t[:], b3r[:])
        ald = cpool.tile([128, NWIN, 8], BF16)     # local al_dst per layer
        ald3 = cpool.tile([128, NWIN, 1], BF16)

        def table_row_epilogue(pool, psum_t, w, layer):
            """psum_t = [128, 144] (h_next|al_src|al_dst) (layers 1,2) or
            [128, 12] (layer3 table).  Write AG input + local al_dst."""
            if layer < 3:
                hrow = pool.tile([128, TW], BF16, tag="hrow")
                nc.gpsimd.memset(hrow[:, 144:TW], 0.0)
                nc.scalar.activation(hrow[:, 0:128], psum_t[:, 0:128],
                                     mybir.ActivationFunctionType.Copy)
                nc.vector.tensor_copy(hrow[:, 128:144].bitcast(FP32), psum_t[:, 128:136])
                nc.scalar.activation(ald[:, w, :], psum_t[:, 136:144],
                                     mybir.ActivationFunctionType.Copy)
                nc.sync.dma_start(agin[w * 128:(w + 1) * 128, :], hrow[:])
            else:
                hrow = pool.tile([128, TW3], FP32, tag="hrow3")
                nc.gpsimd.memset(hrow[:, 12:TW3], 0.0)
                nc.vector.tensor_copy(hrow[:, 0:12], psum_t[:, 0:12])
                nc.scalar.activation(ald3[:, w, :], psum_t[:, 11:12],
                                     mybir.ActivationFunctionType.Copy)
                nc.sync.dma_start(agin3[w * 128:(w + 1) * 128, :], hrow[:])

        # ---------------- layer-1 node matmul: table1 = x @ W1ext ----------------
        with tc.tile_pool(name="mm1", bufs=3) as pool, \
             tc.tile_pool(name="mm1ps", bufs=2, space="PSUM") as pps:
            w1t = [pool.tile([128, 144], FP32, tag=f"w1_{k}", name=f"w1_{k}") for k in range(4)]
            for k in range(4):
                nc.sync.dma_start(w1t[k][:], W1e[k])
            for w in range(NWIN):
                ps = pps.tile([128, 144], FP32, tag="mm1ps")
                for k in range(4):
                    xt = pool.tile([128, 128], FP32, tag="xt")
                    nc.sync.dma_start(xt[:], xT[k, :, w * 128:(w + 1) * 128])
                    nc.tensor.matmul(ps[:], xt[:], w1t[k][:],
                                     start=(k == 0), stop=(k == 3))
                table_row_epilogue(pool, ps, w, 1)

        def allgather(src_ap, dst_ap, pool):
            dummy = pool.tile([1, 1], src_ap.dtype, tag="agdummy")
            nc.gpsimd.dma_start(dummy[:], src_ap[0:1, 0:1])
            nc.gpsimd.collective_compute(
                "AllGather", mybir.AluOpType.bypass,
                replica_groups=[list(range(NC))],
                ins=[src_ap[:]], outs=[dst_ap[:]])

        # ---------------- edge phases ----------------
        def edge_phase(layer):
            EW = ewidth(layer)           # 136 / 11
            FW = 128 if layer < 3 else 10
            tab = table if layer < 3 else table3
            tw = TW if layer < 3 else TW3
            tdt = BF16 if layer < 3 else FP32
            off = 0
            with tc.tile_pool(name=f"ed{layer}", bufs=2) as pool, \
                 tc.tile_pool(name=f"eb{layer}", bufs=1) as bpool, \
                 tc.tile_pool(name=f"eps{layer}", bufs=1, space="PSUM") as pps, \
                 tc.tile_pool(name=f"evs{layer}", bufs=1, space="PSUM") as vps, \
                 tc.tile_pool(name=f"efl{layer}", bufs=1, space="PSUM") as fps:
                for gi, ws in enumerate(groups):
                    cols = int(P[gi].sum()) // 128
                    G = pool.tile([128, cols, tw], tdt, tag="G")
                    ds = bpool.tile([128, cols], FP32, tag="ds")
                    df = bpool.tile([1, cols * 128], BF16, tag="df")
                    nc.sync.dma_start(ds[:], dstS[:, off // 128: off // 128 + cols])
                    nc.sync.dma_start(df[:], dstF[:, off: off + cols * 128])
                    co = 0
                    for s in range(NSUP):
                        pcols = int(P[gi, s]) // 128
                        if pcols == 0:
                            continue
                        nidx = int(P[gi, s])
                        it = pool.tile([128, nidx // 16], I16, tag="it")
                        nc.sync.dma_start(
                            it[:], idxT[:, (off + co * 128) // 16:(off + co * 128 + nidx) // 16])
                        for q0 in range(0, nidx, GCH):
                            q1 = min(q0 + GCH, nidx)
                            nc.gpsimd.dma_gather(
                                G[:, co + q0 // 128: co + q1 // 128, :],
                                tab[s * SUPR:(s + 1) * SUPR, :],
                                it[:, q0 // 16: q1 // 16], q1 - q0, q1 - q0, tw)
                        co += pcols
                    # one-hot builds for the whole group
                    dfr = bpool.tile([128, cols * 128], BF16, tag="dfr")
                    nc.gpsimd.partition_broadcast(dfr[:], df[:])
                    B = bpool.tile([128, cols, 128], BF16, tag="B")
                    nc.vector.tensor_tensor(
                        B[:],
                        iota_row[:].rearrange("p d -> p () d").broadcast_to([128, cols, 128]),
                        ds[:].rearrange("p c -> p c ()").broadcast_to([128, cols, 128]),
                        mybir.AluOpType.is_equal)
                    Bt = bpool.tile([128, cols, 128], BF16, tag="Bt")
                    nc.vector.tensor_scalar(
                        Bt[:], dfr[:].rearrange("p (c e) -> p c e", e=128),
                        iota_col[:], None, mybir.AluOpType.is_equal)
                    # v per edge via per-column broadcast matmuls
                    NH = 8 if layer < 3 else 1
                    vp = vps.tile([128, cols, NH], FP32, tag="vp")
                    aldt = ald if layer < 3 else ald3
                    for c, ents in enumerate(pieces[gi]):
                        if len(ents) == 1:
                            wi = ents[0][0]
                            nc.tensor.matmul(
                                vp[:, c, :], Bt[:, c, :], aldt[:, ws[wi], :],
                                start=True, stop=True)
                        else:
                            for ei, (wi, plo, phi, st, sp) in enumerate(ents):
                                bm = pool.tile([128, 128], BF16, tag="btm", name="btm")
                                nc.vector.memset(bm[:], 0.0)
                                nc.vector.tensor_copy(bm[:, plo:phi], Bt[:, c, plo:phi])
                                nc.tensor.matmul(
                                    vp[:, c, :], bm[:], aldt[:, ws[wi], :],
                                    start=(ei == 0), stop=(ei == len(ents) - 1))
                    # u, t, p
                    if layer < 3:
                        u = G[:, :, 128:144].bitcast(FP32)      # [128, cols, 8]
                    else:
                        u = G[:, :, 10:11]
                    t = pool.tile([128, cols, NH], FP32, tag="t")
                    nc.vector.tensor_tensor(t[:], u, vp[:], mybir.AluOpType.add)
                    lr = pool.tile([128, cols, NH], FP32, tag="lr")
                    nc.vector.tensor_scalar(lr[:], t[:], NEG, None, mybir.AluOpType.mult)
                    nc.vector.tensor_tensor(lr[:], t[:], lr[:], mybir.AluOpType.max)
                    p = pool.tile([128, cols, NH], FP32, tag="p")
                    nc.scalar.activation(p[:], lr[:], mybir.ActivationFunctionType.Exp)
                    # rhs = [msg | p]
                    rhs = pool.tile([128, cols, EW], BF16, tag="rhs")
                    if layer < 3:
                        nc.vector.tensor_tensor(
                            rhs[:, :, 0:128].rearrange("p c (h f) -> p c h f", f=16),
                            G[:, :, 0:128].rearrange("p c (h f) -> p c h f", f=16),
                            p[:].rearrange("p c h -> p c h ()").broadcast_to([128, cols, 8, 16]),
                            mybir.AluOpType.mult)
                        nc.scalar.activation(rhs[:, :, 128:136], p[:],
                                             mybir.ActivationFunctionType.Copy)
                    else:
                        nc.vector.tensor_tensor(
                            rhs[:, :, 0:10],
                            G[:, :, 0:10],
                            p[:].broadcast_to([128, cols, 10]),
                            mybir.AluOpType.mult)
                        nc.scalar.activation(rhs[:, :, 10:11], p[:],
                                             mybir.ActivationFunctionType.Copy)
                    # segment reduce into per-window psum
                    aggs = {}
                    for c, ents in enumerate(pieces[gi]):
                        for (wi, plo, phi, st, sp) in ents:
                            if wi not in aggs:
                                aggs[wi] = pps.tile([128, EW], FP32, tag=f"agg{wi}",
                                                    name=f"agg{wi}")
                            if len(ents) == 1:
                                lhs = B[:, c, :]
                            else:
                                mk = pool.tile([128, 1], FP32, tag="mk", name="mk")
                                if plo == 0:
                                    nc.vector.tensor_scalar(mk[:], iota_col[:], float(phi),
                                                            None, mybir.AluOpType.is_lt)
                                else:
                                    nc.vector.tensor_scalar(mk[:], iota_col[:], float(plo),
                                                            None, mybir.AluOpType.is_ge)
                                bm2 = pool.tile([128, 128], BF16, tag="bm2", name="bm2")
                                nc.vector.tensor_scalar(bm2[:], B[:, c, :], mk[:],
                                                        None, mybir.AluOpType.mult)
                                lhs = bm2[:]
                            nc.tensor.matmul(aggs[wi][:], lhs,
                                             rhs[:, c, :], start=st, stop=sp)
                    # flush each window of this group
                    for wi, w in enumerate(ws):
                        ag = aggs[wi]
                        den = pool.tile([128, NH], FP32, tag="den")
                        nc.vector.reciprocal(den[:], ag[:, FW:EW])
                        z = pool.tile([128, FW], FP32, tag="z")
                        if layer < 3:
                            nc.vector.tensor_tensor(
                                z[:].rearrange("p (h f) -> p h f", f=16),
                                ag[:, 0:128].rearrange("p (h f) -> p h f", f=16),
                                den[:].rearrange("p h -> p h ()").broadcast_to([128, 8, 16]),
                                mybir.AluOpType.mult)
                        else:
                            nc.vector.tensor_tensor(
                                z[:], ag[:, 0:10],
                                den[:].broadcast_to([128, 10]),
                                mybir.AluOpType.mult)
                        bt = (b1t, b2t, b3t)[layer - 1]
                        nc.vector.tensor_tensor(z[:], z[:], bt[:, 0:FW], mybir.AluOpType.add)
                        if layer < 3:
                            # y = ELU(z); yT; table_next = yT.T @ Wnext + c
                            mn = pool.tile([128, FW], FP32, tag="mn")
                            nc.vector.tensor_scalar(mn[:], z[:], 0.0, None, mybir.AluOpType.min)
                            ex = pool.tile([128, FW], FP32, tag="ex")
                            nc.scalar.activation(ex[:], mn[:], mybir.ActivationFunctionType.Exp)
                            y = pool.tile([128, FW], FP32, tag="y")
                            nc.vector.tensor_scalar(y[:], z[:], 0.0, None, mybir.AluOpType.max)
                            nc.vector.tensor_tensor(y[:], y[:], ex[:], mybir.AluOpType.add)
                            nc.vector.tensor_scalar(y[:], y[:], 1.0, None, mybir.AluOpType.subtract)
                            trp = fps.tile([128, 128], FP32, tag="trp")
                            nc.tensor.transpose(trp[:], y[:], ident[:])
                            yT = pool.tile([128, 128], FP32, tag="yT")
                            nc.scalar.activation(yT[:], trp[:], mybir.ActivationFunctionType.Copy)
                            wnt = w2t if layer == 1 else w3t
                            cnt = c2t if layer == 1 else c3t
                            wdt = 144 if layer == 1 else 12
                            tp = fps.tile([128, wdt], FP32, tag="tp")
                            nc.tensor.matmul(tp[:], ones1[:], cnt[:], start=True, stop=False)
                            nc.tensor.matmul(tp[:], yT[:], wnt[:], start=False, stop=True)
                            table_row_epilogue(pool, tp, w, layer + 1)
                        else:
                            # log_softmax over 10 classes
                            m = pool.tile([128, 1], FP32, tag="m")
                            nc.vector.reduce_max(m[:], z[:], axis=mybir.AxisListType.X)
                            tt = pool.tile([128, 10], FP32, tag="tt")
                            nc.vector.tensor_scalar(tt[:], z[:], m[:], None,
                                                    mybir.AluOpType.subtract)
                            et = pool.tile([128, 10], FP32, tag="et")
                            ssum = pool.tile([128, 1], FP32, tag="ssum")
                            nc.scalar.activation(et[:], tt[:],
                                                 mybir.ActivationFunctionType.Exp,
                                                 accum_out=ssum[:])
                            ls = pool.tile([128, 1], FP32, tag="ls")
                            nc.scalar.activation(ls[:], ssum[:],
                                                 mybir.ActivationFunctionType.Ln)
                            ot = pool.tile([128, 10], FP32, tag="ot")
                            nc.vector.tensor_scalar(ot[:], tt[:], ls[:], None,
                                                    mybir.AluOpType.subtract)
                            rows = min(128, NSH - w * 128)
                            if rows > 0:
                                nc.sync.dma_start(
                                    out_ext[w * 128: w * 128 + rows, :], ot[0:rows, :])
                    off += cols * 128

        with tc.tile_pool(name="ag", bufs=1) as agp:
            allgather(agin.ap(), table.ap(), agp)
        edge_phase(1)
        with tc.tile_pool(name="ag2", bufs=1) as agp:
            allgather(agin.ap(), table.ap(), agp)
        edge_phase(2)
        with tc.tile_pool(name="ag3", bufs=1) as agp:
            allgather(agin3.ap(), table3.ap(), agp)
        edge_phase(3)

    nc.compile()
    return nc


def _host_inputs(inputs, struct):
    x = np.asarray(inputs['x'], np.float32)
    s1 = np.asarray(inputs['g1'], np.float32) / np.sqrt(1.0 + BN_EPS)
    s2 = np.asarray(inputs['g2'], np.float32) / np.sqrt(1.0 + BN_EPS)
    be1 = np.asarray(inputs['be1'], np.float32)
    be2 = np.asarray(inputs['be2'], np.float32)
    A1 = np.concatenate([_block_diag_att(np.asarray(inputs['a1_src'])),
                         _block_diag_att(np.asarray(inputs['a1_dst']))], 1)  # [128,16]
    A2 = np.concatenate([_block_diag_att(np.asarray(inputs['a2_src'])),
                         _block_diag_att(np.asarray(inputs['a2_dst']))], 1)
    A3 = np.concatenate([np.asarray(inputs['a3_src']).T,
                         np.asarray(inputs['a3_dst']).T], 1)                 # [10,2]
    W1 = np.asarray(inputs['W1'], np.float32)
    W2 = np.asarray(inputs['W2'], np.float32)
    W3 = np.asarray(inputs['W3'], np.float32)
    W1ext = np.concatenate([W1, W1 @ A1], 1)              # [500,144]
    W2ext = np.concatenate([W2, W2 @ A2], 1)              # [128,144]
    W3ext = np.concatenate([W3, W3 @ A3], 1)              # [128,12]
    W2eff = s1[:, None] * W2ext
    W3eff = s2[:, None] * W3ext
    c2 = (be1 @ W2ext).reshape(1, 144).astype(np.float32)
    c3 = (be2 @ W3ext).reshape(1, 12).astype(np.float32)
    W1p = np.zeros((512, 144), np.float32)
    W1p[:F_IN] = W1ext
    b1 = np.asarray(inputs['b1'], np.float32)
    b2 = np.asarray(inputs['b2'], np.float32)
    b3 = np.asarray(inputs['b3'], np.float32)

    maps = []
    for c in range(NC):
        xs = np.zeros((NSHP, 512), np.float32)
        xs[:NSH, :F_IN] = x[c * NSH:(c + 1) * NSH]
        maps.append({
            "xT": np.ascontiguousarray(xs.T.reshape(4, 128, NSHP)),
            "W1e": np.ascontiguousarray(W1p.reshape(4, 128, 144)),
            "W2e": W2eff, "W3e": W3eff, "c2r": c2, "c3r": c3,
            "b1r": np.tile(b1, (128, 1)), "b2r": np.tile(b2, (128, 1)),
            "b3r": np.tile(b3, (128, 1)),
            "idx": struct['idx_w'][c], "dsts": struct['dst_st'][c],
            "dstf": struct['dst_fl'][c],
        })
    return maps


def kernel(**inputs):
    ei = np.asarray(inputs['edge_index'])
    key = 'prog'
    if key not in _CACHE:
        struct = _preprocess(ei.astype(np.int64))
        nc = _build(struct)
        _CACHE[key] = (struct, nc)
    struct, nc = _CACHE[key]
    in_maps = _host_inputs(inputs, struct)
    t0 = time.time()
    res = run_bass_kernel_spmd(nc, in_maps, list(range(NC)),
                               trace=bool(os.environ.get('KERNEL_TRACE')))
    LAST['wall_s'] = time.time() - t0
    LAST['exec_time_ns'] = res.exec_time_ns
    LAST['res'] = res
    out = np.concatenate([res.results[c]["out"] for c in range(NC)], 0)
    return out.astype(np.float32)


if __name__ == "__main__":
    import reference
    inp = reference.setup_inputs()
    inp = {k: np.asarray(v) for k, v in inp.items()}
    out = kernel(**inp)
    print("kernel out", out.shape, out[:2, :3])


# revision 15
# speedup vs baseline: 1.0133x; 1.0133x over previous
"""GAT (3-layer, 8-head) forward on 8 Trainium2 NeuronCores.

Strategy:
  - Shard nodes (and their incoming edges) across 8 cores: core c owns dst
    nodes [c*12500, (c+1)*12500).
  - Per layer: node features+attention logits live in a replicated DRAM
    "table" (built via AllGather).  Each core gathers table rows for its
    edges' src nodes (dma_gather, int16 idx => 4 table "supers"), computes
    edge softmax weights p = exp(leakyrelu(al_src[s] + al_dst[d])) with NO
    max-shift (logit ranges are small; softmax is shift invariant), and
    segment-sums p*h via one-hot matmuls on the TensorEngine into per-
    128-dst-window PSUM accumulators.  Window flush divides by the summed
    p (denominator), applies bias/ELU/BatchNorm (BN folded into the next
    layer's weights) and computes the next layer's table rows.
"""
import os
import sys
import time
import numpy as np

sys.path.insert(0, '/opt/trn_rl_repo')

import concourse.bass as bass
import concourse.bacc as bacc
import concourse.mybir as mybir
import concourse.tile as tile
from concourse.bass_utils import run_bass_kernel_spmd
import ml_dtypes

# ---------------- problem constants (hardcoded per contract) ----------------
F_IN = 500
HID = 16
HEADS = 8
NCLS = 10
NEG = 0.2
BN_EPS = 1e-5
NC = 8                      # cores


def _config(n=100000, e=1600000):
    g = globals()
    g['N'] = n
    g['E'] = e
    g['NSH'] = n // NC
    g['NWIN'] = (g['NSH'] + 127) // 128
    g['NSHP'] = g['NWIN'] * 128
    g['SUPR'] = (NC * g['NSHP']) // NSUP
    assert g['SUPR'] <= 32767


NSUP = 4                    # table row supers (int16 gather idx limit)
WG = 2                      # windows per group
GCH = 512                   # max idxs per dma_gather
_config()
F1 = HEADS * HID            # 128
TW = 256                    # bf16 table row width (512B): [h(128)|al_src f32(8=16slots)|pad]
TW3 = 64                    # f32 layer-3 table row width (256B)
FP32 = mybir.dt.float32
BF16 = mybir.dt.bfloat16
I16 = mybir.dt.int16

_CACHE = {}
LAST = {}


def _wrap16(a):
    # idx i -> [i%16, i//16]; replicated over the 8 16-partition groups
    n = a.shape[0]
    w = a.reshape(n // 16, 16).T
    return np.tile(w, (8, 1))


def _stripe(a):
    # edge i -> [i%128, i//128]
    n = a.shape[0]
    return a.reshape(n // 128, 128, *a.shape[1:]).swapaxes(0, 1)


def _preprocess(edge_index):
    """Partition + order edges; build per-core gather/dst arrays and the
    (SPMD-uniform) program structure."""
    src = np.concatenate([edge_index[0], np.arange(N, dtype=np.int64)])
    dst = np.concatenate([edge_index[1], np.arange(N, dtype=np.int64)])
    core = dst // NSH
    dloc = dst % NSH
    win = dloc // 128
    wloc = dloc % 128
    trow = (src // NSH) * NSHP + (src % NSH)   # table row (padded shards)
    sup = trow // SUPR
    lidx = trow % SUPR

    # group edges by (core, window, super)
    order = np.lexsort((lidx, sup, win, core))
    core_s, win_s, sup_s = core[order], win[order], sup[order]
    wloc_s, lidx_s = wloc[order], lidx[order]

    # counts[c, w, s]
    key = (core_s * NWIN + win_s) * NSUP + sup_s
    counts = np.bincount(key, minlength=NC * NWIN * NSUP).reshape(NC, NWIN, NSUP)
    L = counts.max(axis=0)                      # [NWIN, NSUP] uniform sub-seg lens
    starts = np.concatenate([[0], np.cumsum(counts.ravel())]).astype(np.int64)

    # groups of WG windows
    groups = [list(range(g, min(g + WG, NWIN))) for g in range(0, NWIN, WG)]
    # P[g][s]: group-super segment length (128-aligned)
    P = np.zeros((len(groups), NSUP), np.int64)
    for gi, ws in enumerate(groups):
        for s in range(NSUP):
            tot = int(L[ws, s].sum())
            P[gi, s] = (tot + 127) // 128 * 128

    EPAD = int(P.sum())

    # per-core flat arrays in processing order
    idx_flat = np.zeros((NC, EPAD), np.int16)
    dst_flat = np.full((NC, EPAD), -1.0, np.float32)
    # per group: cols[c] = list of (w_rel, p_lo, p_hi, start, stop) entries;
    # full-column PE matmuls, masked variants for window-crossing columns
    pieces = []
    for gi, ws in enumerate(groups):
        ncols = int(P[gi].sum()) // 128
        colent = [[] for _ in range(ncols)]
        colbase = 0
        first_seen = set()
        last_ent = {}
        for s in range(NSUP):
            q = 0
            for wi, w in enumerate(ws):
                ln = int(L[w, s])
                for c in range(NC):
                    k = (c * NWIN + w) * NSUP + s
                    cnt = int(counts[c, w, s])
                    a = starts[k]
                    off = int(P[:gi].sum() + P[gi, :s].sum()) + q
                    idx_flat[c, off:off + cnt] = lidx_s[a:a + cnt]
                    dst_flat[c, off:off + cnt] = wloc_s[a:a + cnt]
                rem, qq = ln, q
                while rem > 0:
                    K = min(128 - qq % 128, rem)
                    st = w not in first_seen
                    first_seen.add(w)
                    ent = [wi, qq % 128, qq % 128 + K, st, False]
                    colent[colbase + qq // 128].append(ent)
                    last_ent[w] = ent
                    qq += K
                    rem -= K
                q += ln
            colbase += int(P[gi, s]) // 128
        for w in ws:
            last_ent[w][4] = True
        for ents in colent:
            assert len(ents) <= 2, "column spans >2 windows"
        pieces.append(colent)

    # permuted edge order (per core) for assembling idx arrays done above;
    # build wrapped/striped host tensors
    idx_w = np.stack([_wrap16(idx_flat[c]) for c in range(NC)])        # [NC,128,EPAD/16]
    dst_st = np.stack([_stripe(dst_flat[c]) for c in range(NC)])       # [NC,128,EPAD/128]
    dst_fl = dst_flat.reshape(NC, 1, EPAD)

    return dict(groups=groups, P=P, EPAD=EPAD, pieces=pieces,
                idx_w=idx_w.astype(np.int16), dst_st=np.ascontiguousarray(dst_st),
                dst_fl=np.ascontiguousarray(dst_fl).astype(ml_dtypes.bfloat16))


def _block_diag_att(a):
    # a: [H, C] -> A [H*C, H] with A[h*C+c, h] = a[h, c]
    Hh, C = a.shape
    A = np.zeros((Hh * C, Hh), np.float32)
    for h in range(Hh):
        A[h * C:(h + 1) * C, h] = a[h]
    return A


def _build(struct):
    groups, P, EPAD, pieces = struct['groups'], struct['P'], struct['EPAD'], struct['pieces']
    NGRP = len(groups)

    nc = bacc.Bacc("TRN2", target_bir_lowering=False, debug=False, num_devices=NC)

    # ---- parameters ----
    xT = nc.declare_dram_parameter("xT", [4, 128, NSHP], FP32, isOutput=False)
    W1e = nc.declare_dram_parameter("W1e", [4, 128, 144], FP32, isOutput=False)
    W2e = nc.declare_dram_parameter("W2e", [128, 144], FP32, isOutput=False)
    W3e = nc.declare_dram_parameter("W3e", [128, 12], FP32, isOutput=False)
    c2r = nc.declare_dram_parameter("c2r", [1, 144], FP32, isOutput=False)
    c3r = nc.declare_dram_parameter("c3r", [1, 12], FP32, isOutput=False)
    b1r = nc.declare_dram_parameter("b1r", [128, 128], FP32, isOutput=False)
    b2r = nc.declare_dram_parameter("b2r", [128, 128], FP32, isOutput=False)
    b3r = nc.declare_dram_parameter("b3r", [128, 10], FP32, isOutput=False)
    idxT = nc.declare_dram_parameter("idx", [128, EPAD // 16], I16, isOutput=False)
    dstS = nc.declare_dram_parameter("dsts", [128, EPAD // 128], FP32, isOutput=False)
    dstF = nc.declare_dram_parameter("dstf", [1, EPAD], BF16, isOutput=False)
    out_ext = nc.declare_dram_parameter("out", [NSH, 10], FP32, isOutput=True)

    # ---- internal DRAM ----
    table = nc.dram_tensor("table", [NC * NSHP, TW], BF16, addr_space="Shared")
    table3 = nc.dram_tensor("table3", [NC * NSHP, TW3], FP32, addr_space="Shared")
    agin = nc.dram_tensor("agin", [NSHP, TW], BF16)
    agin3 = nc.dram_tensor("agin3", [NSHP, TW3], FP32)

    def ewidth(layer):   # rhs width: msg | p
        return 136 if layer < 3 else 11

    with tile.TileContext(nc, num_cores=NC) as tc, \
         tc.tile_pool(name="consts", bufs=1) as cpool:
        # constants
        iota_col = cpool.tile([128, 1], FP32)
        nc.gpsimd.iota(iota_col[:], [[0, 1]], base=0, channel_multiplier=1,
                       allow_small_or_imprecise_dtypes=True)
        iota_row = cpool.tile([128, 128], FP32)
        nc.gpsimd.iota(iota_row[:], [[1, 128]], base=0, channel_multiplier=0,
                       allow_small_or_imprecise_dtypes=True)
        ident = cpool.tile([128, 128], FP32)
        nc.vector.tensor_scalar(ident[:], iota_row[:], iota_col[:], None,
                                mybir.AluOpType.is_equal)
        ones1 = cpool.tile([1, 128], FP32)
        nc.gpsimd.memset(ones1[:], 1.0)
        w2t = cpool.tile([128, 144], FP32)
        nc.sync.dma_start(w2t[:], W2e[:])
        w3t = cpool.tile([128, 12], FP32)
        nc.sync.dma_start(w3t[:], W3e[:])
        c2t = cpool.tile([1, 144], FP32)
        nc.sync.dma_start(c2t[:], c2r[:])
        c3t = cpool.tile([1, 12], FP32)
        nc.sync.dma_start(c3t[:], c3r[:])
        b1t = cpool.tile([128, 128], FP32)
        nc.sync.dma_start(b1t[:], b1r[:])
        b2t = cpool.tile([128, 128], FP32)
        nc.sync.dma_start(b2t[:], b2r[:])
        b3t = cpool.tile([128, 10], FP32)
        nc.sync.dma_start(b3t[:], b3r[:])
        ald = cpool.tile([128, NWIN, 8], BF16)     # local al_dst per layer
        ald3 = cpool.tile([128, NWIN, 1], BF16)

        def table_row_epilogue(pool, psum_t, w, layer):
            """psum_t = [128, 144] (h_next|al_src|al_dst) (layers 1,2) or
            [128, 12] (layer3 table).  Write AG input + local al_dst."""
            if layer < 3:
                hrow = pool.tile([128, TW], BF16, tag="hrow")
                nc.gpsimd.memset(hrow[:, 144:TW], 0.0)
                nc.scalar.activation(hrow[:, 0:128], psum_t[:, 0:128],
                                     mybir.ActivationFunctionType.Copy)
                nc.vector.tensor_copy(hrow[:, 128:144].bitcast(FP32), psum_t[:, 128:136])
                nc.scalar.activation(ald[:, w, :], psum_t[:, 136:144],
                                     mybir.ActivationFunctionType.Copy)
                nc.sync.dma_start(agin[w * 128:(w + 1) * 128, :], hrow[:])
            else:
                hrow = pool.tile([128, TW3], FP32, tag="hrow3")
                nc.gpsimd.memset(hrow[:, 12:TW3], 0.0)
                nc.vector.tensor_copy(hrow[:, 0:12], psum_t[:, 0:12])
                nc.scalar.activation(ald3[:, w, :], psum_t[:, 11:12],
                                     mybir.ActivationFunctionType.Copy)
                nc.sync.dma_start(agin3[w * 128:(w + 1) * 128, :], hrow[:])

        # ---------------- layer-1 node matmul: table1 = x @ W1ext ----------------
        with tc.tile_pool(name="mm1", bufs=3) as pool, \
             tc.tile_pool(name="mm1ps", bufs=2, space="PSUM") as pps:
            w1t = [pool.tile([128, 144], FP32, tag=f"w1_{k}", name=f"w1_{k}") for k in range(4)]
            for k in range(4):
                nc.sync.dma_start(w1t[k][:], W1e[k])
            for w in range(NWIN):
                ps = pps.tile([128, 144], FP32, tag="mm1ps")
                for k in range(4):
                    xt = pool.tile([128, 128], FP32, tag="xt")
                    nc.sync.dma_start(xt[:], xT[k, :, w * 128:(w + 1) * 128])
                    nc.tensor.matmul(ps[:], xt[:], w1t[k][:],
                                     start=(k == 0), stop=(k == 3))
                table_row_epilogue(pool, ps, w, 1)

        def allgather(src_ap, dst_ap, pool):
            dummy = pool.tile([1, 1], src_ap.dtype, tag="agdummy")
            nc.gpsimd.dma_start(dummy[:], src_ap[0:1, 0:1])
            nc.gpsimd.collective_compute(
                "AllGather", mybir.AluOpType.bypass,
                replica_groups=[list(range(NC))],
                ins=[src_ap[:]], outs=[dst_ap[:]])

        # ---------------- edge phases ----------------
        def edge_phase(layer):
            EW = ewidth(layer)           # 136 / 11
            FW = 128 if layer < 3 else 10
            tab = table if layer < 3 else table3
            tw = TW if layer < 3 else TW3
            tdt = BF16 if layer < 3 else FP32
            off = 0
            with tc.tile_pool(name=f"ed{layer}", bufs=2) as pool, \
                 tc.tile_pool(name=f"eb{layer}", bufs=1) as bpool, \
                 tc.tile_pool(name=f"eps{layer}", bufs=1, space="PSUM") as pps, \
                 tc.tile_pool(name=f"evs{layer}", bufs=2, space="PSUM") as vps, \
                 tc.tile_pool(name=f"efl{layer}", bufs=2, space="PSUM") as fps:
                for gi, ws in enumerate(groups):
                    cols = int(P[gi].sum()) // 128
                    G = pool.tile([128, cols, tw], tdt, tag="G")
                    ds = bpool.tile([128, cols], FP32, tag="ds")
                    df = bpool.tile([1, cols * 128], BF16, tag="df")
                    nc.sync.dma_start(ds[:], dstS[:, off // 128: off // 128 + cols])
                    nc.sync.dma_start(df[:], dstF[:, off: off + cols * 128])
                    co = 0
                    for s in range(NSUP):
                        pcols = int(P[gi, s]) // 128
                        if pcols == 0:
                            continue
                        nidx = int(P[gi, s])
                        it = pool.tile([128, nidx // 16], I16, tag="it")
                        nc.sync.dma_start(
                            it[:], idxT[:, (off + co * 128) // 16:(off + co * 128 + nidx) // 16])
                        for q0 in range(0, nidx, GCH):
                            q1 = min(q0 + GCH, nidx)
                            nc.gpsimd.dma_gather(
                                G[:, co + q0 // 128: co + q1 // 128, :],
                                tab[s * SUPR:(s + 1) * SUPR, :],
                                it[:, q0 // 16: q1 // 16], q1 - q0, q1 - q0, tw)
                        co += pcols
                    # one-hot builds for the whole group
                    NOVB = bool(os.environ.get('KERNEL_NOVB'))
                    dfr = bpool.tile([128, cols * 128], BF16, tag="dfr")
                    if not NOVB:
                        nc.gpsimd.partition_broadcast(dfr[:], df[:])
                    B = bpool.tile([128, cols, 128], BF16, tag="B")
                    nc.vector.tensor_tensor(
                        B[:],
                        iota_row[:].rearrange("p d -> p () d").broadcast_to([128, cols, 128]),
                        ds[:].rearrange("p c -> p c ()").broadcast_to([128, cols, 128]),
                        mybir.AluOpType.is_equal)
                    Bt = bpool.tile([128, cols, 128], BF16, tag="Bt")
                    if not NOVB:
                        nc.vector.tensor_scalar(
                            Bt[:], dfr[:].rearrange("p (c e) -> p c e", e=128),
                            iota_col[:], None, mybir.AluOpType.is_equal)
                    # v per edge via per-column broadcast matmuls
                    NH = 8 if layer < 3 else 1
                    vp = vps.tile([128, cols, NH], FP32, tag="vp")
                    aldt = ald if layer < 3 else ald3
                    for c, ents in enumerate([] if NOVB else pieces[gi]):
                        if len(ents) == 1:
                            wi = ents[0][0]
                            nc.tensor.matmul(
                                vp[:, c, :], Bt[:, c, :], aldt[:, ws[wi], :],
                                start=True, stop=True)
                        else:
                            for ei, (wi, plo, phi, st, sp) in enumerate(ents):
                                bm = pool.tile([128, 128], BF16, tag="btm", name="btm")
                                nc.vector.memset(bm[:], 0.0)
                                nc.vector.tensor_copy(bm[:, plo:phi], Bt[:, c, plo:phi])
                                nc.tensor.matmul(
                                    vp[:, c, :], bm[:], aldt[:, ws[wi], :],
                                    start=(ei == 0), stop=(ei == len(ents) - 1))
                    # u, t, p
                    if layer < 3:
                        u = G[:, :, 128:144].bitcast(FP32)      # [128, cols, 8]
                    else:
                        u = G[:, :, 10:11]
                    t = pool.tile([128, cols, NH], FP32, tag="t")
                    if NOVB:
                        nc.vector.tensor_copy(t[:], u)
                    else:
                        nc.vector.tensor_tensor(t[:], u, vp[:], mybir.AluOpType.add)
                    lr = pool.tile([128, cols, NH], FP32, tag="lr")
                    nc.vector.tensor_scalar(lr[:], t[:], NEG, None, mybir.AluOpType.mult)
                    nc.vector.tensor_tensor(lr[:], t[:], lr[:], mybir.AluOpType.max)
                    p = pool.tile([128, cols, NH], FP32, tag="p")
                    nc.scalar.activation(p[:], lr[:], mybir.ActivationFunctionType.Exp)
                    # rhs = [msg | p]
                    rhs = pool.tile([128, cols, EW], BF16, tag="rhs")
                    if layer < 3:
                        nc.vector.tensor_tensor(
                            rhs[:, :, 0:128].rearrange("p c (h f) -> p c h f", f=16),
                            G[:, :, 0:128].rearrange("p c (h f) -> p c h f", f=16),
                            p[:].rearrange("p c h -> p c h ()").broadcast_to([128, cols, 8, 16]),
                            mybir.AluOpType.mult)
                        nc.scalar.activation(rhs[:, :, 128:136], p[:],
                                             mybir.ActivationFunctionType.Copy)
                    else:
                        nc.vector.tensor_tensor(
                            rhs[:, :, 0:10],
                            G[:, :, 0:10],
                            p[:].broadcast_to([128, cols, 10]),
                            mybir.AluOpType.mult)
                        nc.scalar.activation(rhs[:, :, 10:11], p[:],
                                             mybir.ActivationFunctionType.Copy)
                    # segment reduce into per-window psum
                    aggs = {}
                    for c, ents in enumerate(pieces[gi]):
                        for (wi, plo, phi, st, sp) in ents:
                            if wi not in aggs:
                                aggs[wi] = pps.tile([128, EW], FP32, tag=f"agg{wi}",
                                                    name=f"agg{wi}")
                            if len(ents) == 1:
                                lhs = B[:, c, :]
                            else:
                                mk = pool.tile([128, 1], FP32, tag="mk", name="mk")
                                if plo == 0:
                                    nc.vector.tensor_scalar(mk[:], iota_col[:], float(phi),
                                                            None, mybir.AluOpType.is_lt)
                                else:
                                    nc.vector.tensor_scalar(mk[:], iota_col[:], float(plo),
                                                            None, mybir.AluOpType.is_ge)
                                bm2 = pool.tile([128, 128], BF16, tag="bm2", name="bm2")
                                nc.vector.tensor_scalar(bm2[:], B[:, c, :], mk[:],
                                                        None, mybir.AluOpType.mult)
                                lhs = bm2[:]
                            nc.tensor.matmul(aggs[wi][:], lhs,
                                             rhs[:, c, :], start=st, stop=sp)
                    # flush each window of this group
                    for wi, w in enumerate(ws):
                        ag = aggs[wi]
                        den = pool.tile([128, NH], FP32, tag="den")
                        nc.vector.reciprocal(den[:], ag[:, FW:EW])
                        z = pool.tile([128, FW], FP32, tag="z")
                        if layer < 3:
                            nc.vector.tensor_tensor(
                                z[:].rearrange("p (h f) -> p h f", f=16),
                                ag[:, 0:128].rearrange("p (h f) -> p h f", f=16),
                                den[:].rearrange("p h -> p h ()").broadcast_to([128, 8, 16]),
                                mybir.AluOpType.mult)
                        else:
                            nc.vector.tensor_tensor(
                                z[:], ag[:, 0:10],
                                den[:].broadcast_to([128, 10]),
                                mybir.AluOpType.mult)
                        bt = (b1t, b2t, b3t)[layer - 1]
                        nc.vector.tensor_tensor(z[:], z[:], bt[:, 0:FW], mybir.AluOpType.add)
                        if layer < 3:
                            # y = ELU(z); yT; table_next = yT.T @ Wnext + c
                            mn = pool.tile([128, FW], FP32, tag="mn")
                            nc.vector.tensor_scalar(mn[:], z[:], 0.0, None, mybir.AluOpType.min)
                            ex = pool.tile([128, FW], FP32, tag="ex")
                            nc.scalar.activation(ex[:], mn[:], mybir.ActivationFunctionType.Exp)
                            y = pool.tile([128, FW], FP32, tag="y")
                            nc.vector.tensor_scalar(y[:], z[:], 0.0, None, mybir.AluOpType.max)
                            nc.vector.tensor_tensor(y[:], y[:], ex[:], mybir.AluOpType.add)
                            nc.vector.tensor_scalar(y[:], y[:], 1.0, None, mybir.AluOpType.subtract)
                            trp = fps.tile([128, 128], FP32, tag="trp")
                            nc.tensor.transpose(trp[:], y[:], ident[:])
                            yT = pool.tile([128, 128], FP32, tag="yT")
                            nc.scalar.activation(yT[:], trp[:], mybir.ActivationFunctionType.Copy)
                            wnt = w2t if layer == 1 else w3t
                            cnt = c2t if layer == 1 else c3t
                            wdt = 144 if layer == 1 else 12
                            tp = fps.tile([128, wdt], FP32, tag="tp")
                            nc.tensor.matmul(tp[:], ones1[:], cnt[:], start=True, stop=False)
                            nc.tensor.matmul(tp[:], yT[:], wnt[:], start=False, stop=True)
                            table_row_epilogue(pool, tp, w, layer + 1)
                        else:
                            # log_softmax over 10 classes
                            m = pool.tile([128, 1], FP32, tag="m")
                            nc.vector.reduce_max(m[:], z[:], axis=mybir.AxisListType.X)
                            tt = pool.tile([128, 10], FP32, tag="tt")
                            nc.vector.tensor_scalar(tt[:], z[:], m[:], None,
                                                    mybir.AluOpType.subtract)
                            et = pool.tile([128, 10], FP32, tag="et")
                            ssum = pool.tile([128, 1], FP32, tag="ssum")
                            nc.scalar.activation(et[:], tt[:],
                                                 mybir.ActivationFunctionType.Exp,
                                                 accum_out=ssum[:])
                            ls = pool.tile([128, 1], FP32, tag="ls")
                            nc.scalar.activation(ls[:], ssum[:],
                                                 mybir.ActivationFunctionType.Ln)
                            ot = pool.tile([128, 10], FP32, tag="ot")
                            nc.vector.tensor_scalar(ot[:], tt[:], ls[:], None,
                                                    mybir.AluOpType.subtract)
                            rows = min(128, NSH - w * 128)
                            if rows > 0:
                                nc.sync.dma_start(
                                    out_ext[w * 128: w * 128 + rows, :], ot[0:rows, :])
                    off += cols * 128

        with tc.tile_pool(name="ag", bufs=1) as agp:
            allgather(agin.ap(), table.ap(), agp)
        edge_phase(1)
        with tc.tile_pool(name="ag2", bufs=1) as agp:
            allgather(agin.ap(), table.ap(), agp)
        edge_phase(2)
        with tc.tile_pool(name="ag3", bufs=1) as agp:
            allgather(agin3.ap(), table3.ap(), agp)
        edge_phase(3)

    nc.compile()
    return nc


def _host_inputs(inputs, struct):
    x = np.asarray(inputs['x'], np.float32)
    s1 = np.asarray(inputs['g1'], np.float32) / np.sqrt(1.0 + BN_EPS)
    s2 = np.asarray(inputs['g2'], np.float32) / np.sqrt(1.0 + BN_EPS)
    be1 = np.asarray(inputs['be1'], np.float32)
    be2 = np.asarray(inputs['be2'], np.float32)
    A1 = np.concatenate([_block_diag_att(np.asarray(inputs['a1_src'])),
                         _block_diag_att(np.asarray(inputs['a1_dst']))], 1)  # [128,16]
    A2 = np.concatenate([_block_diag_att(np.asarray(inputs['a2_src'])),
                         _block_diag_att(np.asarray(inputs['a2_dst']))], 1)
    A3 = np.concatenate([np.asarray(inputs['a3_src']).T,
                         np.asarray(inputs['a3_dst']).T], 1)                 # [10,2]
    W1 = np.asarray(inputs['W1'], np.float32)
    W2 = np.asarray(inputs['W2'], np.float32)
    W3 = np.asarray(inputs['W3'], np.float32)
    W1ext = np.concatenate([W1, W1 @ A1], 1)              # [500,144]
    W2ext = np.concatenate([W2, W2 @ A2], 1)              # [128,144]
    W3ext = np.concatenate([W3, W3 @ A3], 1)              # [128,12]
    W2eff = s1[:, None] * W2ext
    W3eff = s2[:, None] * W3ext
    c2 = (be1 @ W2ext).reshape(1, 144).astype(np.float32)
    c3 = (be2 @ W3ext).reshape(1, 12).astype(np.float32)
    W1p = np.zeros((512, 144), np.float32)
    W1p[:F_IN] = W1ext
    b1 = np.asarray(inputs['b1'], np.float32)
    b2 = np.asarray(inputs['b2'], np.float32)
    b3 = np.asarray(inputs['b3'], np.float32)

    maps = []
    for c in range(NC):
        xs = np.zeros((NSHP, 512), np.float32)
        xs[:NSH, :F_IN] = x[c * NSH:(c + 1) * NSH]
        maps.append({
            "xT": np.ascontiguousarray(xs.T.reshape(4, 128, NSHP)),
            "W1e": np.ascontiguousarray(W1p.reshape(4, 128, 144)),
            "W2e": W2eff, "W3e": W3eff, "c2r": c2, "c3r": c3,
            "b1r": np.tile(b1, (128, 1)), "b2r": np.tile(b2, (128, 1)),
            "b3r": np.tile(b3, (128, 1)),
            "idx": struct['idx_w'][c], "dsts": struct['dst_st'][c],
            "dstf": struct['dst_fl'][c],
        })
    return maps


def kernel(**inputs):
    ei = np.asarray(inputs['edge_index'])
    key = 'prog'
    if key not in _CACHE:
        struct = _preprocess(ei.astype(np.int64))
        nc = _build(struct)
        _CACHE[key] = (struct, nc)
    struct, nc = _CACHE[key]
    in_maps = _host_inputs(inputs, struct)
    t0 = time.time()
    res = run_bass_kernel_spmd(nc, in_maps, list(range(NC)),
                               trace=bool(os.environ.get('KERNEL_TRACE')))
    LAST['wall_s'] = time.time() - t0
    LAST['exec_time_ns'] = res.exec_time_ns
    LAST['res'] = res
    out = np.concatenate([res.results[c]["out"] for c in range(NC)], 0)
    return out.astype(np.float32)


if __name__ == "__main__":
    import reference
    inp = reference.setup_inputs()
    inp = {k: np.asarray(v) for k, v in inp.items()}
    out = kernel(**inp)
    print("kernel out", out.shape, out[:2, :3])


# revision 16
# speedup vs baseline: 1.0264x; 1.0130x over previous
"""GAT (3-layer, 8-head) forward on 8 Trainium2 NeuronCores.

Strategy:
  - Shard nodes (and their incoming edges) across 8 cores: core c owns dst
    nodes [c*12500, (c+1)*12500).
  - Per layer: node features+attention logits live in a replicated DRAM
    "table" (built via AllGather).  Each core gathers table rows for its
    edges' src nodes (dma_gather, int16 idx => 4 table "supers"), computes
    edge softmax weights p = exp(leakyrelu(al_src[s] + al_dst[d])) with NO
    max-shift (logit ranges are small; softmax is shift invariant), and
    segment-sums p*h via one-hot matmuls on the TensorEngine into per-
    128-dst-window PSUM accumulators.  Window flush divides by the summed
    p (denominator), applies bias/ELU/BatchNorm (BN folded into the next
    layer's weights) and computes the next layer's table rows.
"""
import os
import sys
import time
import numpy as np

sys.path.insert(0, '/opt/trn_rl_repo')

import concourse.bass as bass
import concourse.bacc as bacc
import concourse.mybir as mybir
import concourse.tile as tile
from concourse.bass_utils import run_bass_kernel_spmd
import ml_dtypes

# ---------------- problem constants (hardcoded per contract) ----------------
F_IN = 500
HID = 16
HEADS = 8
NCLS = 10
NEG = 0.2
BN_EPS = 1e-5
NC = 8                      # cores


def _config(n=100000, e=1600000):
    g = globals()
    g['N'] = n
    g['E'] = e
    g['NSH'] = n // NC
    g['NWIN'] = (g['NSH'] + 127) // 128
    g['NSHP'] = g['NWIN'] * 128
    g['SUPR'] = (NC * g['NSHP']) // NSUP
    assert g['SUPR'] <= 32767


NSUP = 4                    # table row supers (int16 gather idx limit)
WG = 2                      # windows per group
GCH = 512                   # max idxs per dma_gather
_config()
F1 = HEADS * HID            # 128
TW = 256                    # bf16 table row width (512B): [h(128)|al_src f32(8=16slots)|pad]
TW3 = 64                    # f32 layer-3 table row width (256B)
FP32 = mybir.dt.float32
BF16 = mybir.dt.bfloat16
I16 = mybir.dt.int16

_CACHE = {}
LAST = {}


def _wrap16(a):
    # idx i -> [i%16, i//16]; replicated over the 8 16-partition groups
    n = a.shape[0]
    w = a.reshape(n // 16, 16).T
    return np.tile(w, (8, 1))


def _stripe(a):
    # edge i -> [i%128, i//128]
    n = a.shape[0]
    return a.reshape(n // 128, 128, *a.shape[1:]).swapaxes(0, 1)


def _preprocess(edge_index):
    """Partition + order edges; build per-core gather/dst arrays and the
    (SPMD-uniform) program structure."""
    src = np.concatenate([edge_index[0], np.arange(N, dtype=np.int64)])
    dst = np.concatenate([edge_index[1], np.arange(N, dtype=np.int64)])
    core = dst // NSH
    dloc = dst % NSH
    win = dloc // 128
    wloc = dloc % 128
    trow = (src // NSH) * NSHP + (src % NSH)   # table row (padded shards)
    sup = trow // SUPR
    lidx = trow % SUPR

    # group edges by (core, window, super)
    order = np.lexsort((lidx, sup, win, core))
    core_s, win_s, sup_s = core[order], win[order], sup[order]
    wloc_s, lidx_s = wloc[order], lidx[order]

    # counts[c, w, s]
    key = (core_s * NWIN + win_s) * NSUP + sup_s
    counts = np.bincount(key, minlength=NC * NWIN * NSUP).reshape(NC, NWIN, NSUP)
    L = counts.max(axis=0)                      # [NWIN, NSUP] uniform sub-seg lens
    starts = np.concatenate([[0], np.cumsum(counts.ravel())]).astype(np.int64)

    # groups of WG windows
    groups = [list(range(g, min(g + WG, NWIN))) for g in range(0, NWIN, WG)]
    # P[g][s]: group-super segment length (128-aligned)
    P = np.zeros((len(groups), NSUP), np.int64)
    for gi, ws in enumerate(groups):
        for s in range(NSUP):
            tot = int(L[ws, s].sum())
            P[gi, s] = (tot + 127) // 128 * 128

    EPAD = int(P.sum())

    # per-core flat arrays in processing order
    idx_flat = np.zeros((NC, EPAD), np.int16)
    dst_flat = np.full((NC, EPAD), -1.0, np.float32)
    # per group: cols[c] = list of (w_rel, p_lo, p_hi, start, stop) entries;
    # full-column PE matmuls, masked variants for window-crossing columns
    pieces = []
    for gi, ws in enumerate(groups):
        ncols = int(P[gi].sum()) // 128
        colent = [[] for _ in range(ncols)]
        colbase = 0
        first_seen = set()
        last_ent = {}
        for s in range(NSUP):
            q = 0
            for wi, w in enumerate(ws):
                ln = int(L[w, s])
                for c in range(NC):
                    k = (c * NWIN + w) * NSUP + s
                    cnt = int(counts[c, w, s])
                    a = starts[k]
                    off = int(P[:gi].sum() + P[gi, :s].sum()) + q
                    idx_flat[c, off:off + cnt] = lidx_s[a:a + cnt]
                    dst_flat[c, off:off + cnt] = wloc_s[a:a + cnt]
                rem, qq = ln, q
                while rem > 0:
                    K = min(128 - qq % 128, rem)
                    st = w not in first_seen
                    first_seen.add(w)
                    ent = [wi, qq % 128, qq % 128 + K, st, False]
                    colent[colbase + qq // 128].append(ent)
                    last_ent[w] = ent
                    qq += K
                    rem -= K
                q += ln
            colbase += int(P[gi, s]) // 128
        for w in ws:
            last_ent[w][4] = True
        for ents in colent:
            assert len(ents) <= 2, "column spans >2 windows"
        pieces.append(colent)

    # permuted edge order (per core) for assembling idx arrays done above;
    # build wrapped/striped host tensors
    idx_w = np.stack([_wrap16(idx_flat[c]) for c in range(NC)])        # [NC,128,EPAD/16]
    dst_st = np.stack([_stripe(dst_flat[c]) for c in range(NC)])       # [NC,128,EPAD/128]
    dst_fl = dst_flat.reshape(NC, 1, EPAD)

    return dict(groups=groups, P=P, EPAD=EPAD, pieces=pieces,
                idx_w=idx_w.astype(np.int16), dst_st=np.ascontiguousarray(dst_st),
                dst_fl=np.ascontiguousarray(dst_fl).astype(ml_dtypes.bfloat16))


def _block_diag_att(a):
    # a: [H, C] -> A [H*C, H] with A[h*C+c, h] = a[h, c]
    Hh, C = a.shape
    A = np.zeros((Hh * C, Hh), np.float32)
    for h in range(Hh):
        A[h * C:(h + 1) * C, h] = a[h]
    return A


def _build(struct):
    groups, P, EPAD, pieces = struct['groups'], struct['P'], struct['EPAD'], struct['pieces']
    NGRP = len(groups)

    nc = bacc.Bacc("TRN2", target_bir_lowering=False, debug=False, num_devices=NC)

    # ---- parameters ----
    xT = nc.declare_dram_parameter("xT", [4, 128, NSHP], FP32, isOutput=False)
    W1e = nc.declare_dram_parameter("W1e", [4, 128, 144], FP32, isOutput=False)
    W2e = nc.declare_dram_parameter("W2e", [128, 144], FP32, isOutput=False)
    W3e = nc.declare_dram_parameter("W3e", [128, 12], FP32, isOutput=False)
    c2r = nc.declare_dram_parameter("c2r", [1, 144], FP32, isOutput=False)
    c3r = nc.declare_dram_parameter("c3r", [1, 12], FP32, isOutput=False)
    b1r = nc.declare_dram_parameter("b1r", [128, 128], FP32, isOutput=False)
    b2r = nc.declare_dram_parameter("b2r", [128, 128], FP32, isOutput=False)
    b3r = nc.declare_dram_parameter("b3r", [128, 10], FP32, isOutput=False)
    idxT = nc.declare_dram_parameter("idx", [128, EPAD // 16], I16, isOutput=False)
    dstS = nc.declare_dram_parameter("dsts", [128, EPAD // 128], FP32, isOutput=False)
    dstF = nc.declare_dram_parameter("dstf", [1, EPAD], BF16, isOutput=False)
    out_ext = nc.declare_dram_parameter("out", [NSH, 10], FP32, isOutput=True)

    # ---- internal DRAM ----
    table = nc.dram_tensor("table", [NC * NSHP, TW], BF16, addr_space="Shared")
    table3 = nc.dram_tensor("table3", [NC * NSHP, TW3], FP32, addr_space="Shared")
    agin = nc.dram_tensor("agin", [NSHP, TW], BF16)
    agin3 = nc.dram_tensor("agin3", [NSHP, TW3], FP32)

    def ewidth(layer):   # rhs width: msg | p
        return 136 if layer < 3 else 11

    with tile.TileContext(nc, num_cores=NC) as tc, \
         tc.tile_pool(name="consts", bufs=1) as cpool:
        # constants
        iota_col = cpool.tile([128, 1], FP32)
        nc.gpsimd.iota(iota_col[:], [[0, 1]], base=0, channel_multiplier=1,
                       allow_small_or_imprecise_dtypes=True)
        iota_row = cpool.tile([128, 128], FP32)
        nc.gpsimd.iota(iota_row[:], [[1, 128]], base=0, channel_multiplier=0,
                       allow_small_or_imprecise_dtypes=True)
        ident = cpool.tile([128, 128], FP32)
        nc.vector.tensor_scalar(ident[:], iota_row[:], iota_col[:], None,
                                mybir.AluOpType.is_equal)
        ones1 = cpool.tile([1, 128], FP32)
        nc.gpsimd.memset(ones1[:], 1.0)
        w2t = cpool.tile([128, 144], FP32)
        nc.sync.dma_start(w2t[:], W2e[:])
        w3t = cpool.tile([128, 12], FP32)
        nc.sync.dma_start(w3t[:], W3e[:])
        c2t = cpool.tile([1, 144], FP32)
        nc.sync.dma_start(c2t[:], c2r[:])
        c3t = cpool.tile([1, 12], FP32)
        nc.sync.dma_start(c3t[:], c3r[:])
        b1t = cpool.tile([128, 128], FP32)
        nc.sync.dma_start(b1t[:], b1r[:])
        b2t = cpool.tile([128, 128], FP32)
        nc.sync.dma_start(b2t[:], b2r[:])
        b3t = cpool.tile([128, 10], FP32)
        nc.sync.dma_start(b3t[:], b3r[:])
        ald = cpool.tile([128, NWIN, 8], BF16)     # local al_dst per layer
        ald3 = cpool.tile([128, NWIN, 1], BF16)

        def table_row_epilogue(pool, psum_t, w, layer):
            """psum_t = [128, 144] (h_next|al_src|al_dst) (layers 1,2) or
            [128, 12] (layer3 table).  Write AG input + local al_dst."""
            if layer < 3:
                hrow = pool.tile([128, TW], BF16, tag="hrow")
                nc.gpsimd.memset(hrow[:, 144:TW], 0.0)
                nc.scalar.activation(hrow[:, 0:128], psum_t[:, 0:128],
                                     mybir.ActivationFunctionType.Copy)
                nc.vector.tensor_copy(hrow[:, 128:144].bitcast(FP32), psum_t[:, 128:136])
                nc.scalar.activation(ald[:, w, :], psum_t[:, 136:144],
                                     mybir.ActivationFunctionType.Copy)
                nc.sync.dma_start(agin[w * 128:(w + 1) * 128, :], hrow[:])
            else:
                hrow = pool.tile([128, TW3], FP32, tag="hrow3")
                nc.gpsimd.memset(hrow[:, 12:TW3], 0.0)
                nc.vector.tensor_copy(hrow[:, 0:12], psum_t[:, 0:12])
                nc.scalar.activation(ald3[:, w, :], psum_t[:, 11:12],
                                     mybir.ActivationFunctionType.Copy)
                nc.sync.dma_start(agin3[w * 128:(w + 1) * 128, :], hrow[:])

        # ---------------- layer-1 node matmul: table1 = x @ W1ext ----------------
        with tc.tile_pool(name="mm1", bufs=3) as pool, \
             tc.tile_pool(name="mm1ps", bufs=2, space="PSUM") as pps:
            w1t = [pool.tile([128, 144], FP32, tag=f"w1_{k}", name=f"w1_{k}") for k in range(4)]
            for k in range(4):
                nc.sync.dma_start(w1t[k][:], W1e[k])
            for w in range(NWIN):
                ps = pps.tile([128, 144], FP32, tag="mm1ps")
                for k in range(4):
                    xt = pool.tile([128, 128], FP32, tag="xt")
                    nc.sync.dma_start(xt[:], xT[k, :, w * 128:(w + 1) * 128])
                    nc.tensor.matmul(ps[:], xt[:], w1t[k][:],
                                     start=(k == 0), stop=(k == 3))
                table_row_epilogue(pool, ps, w, 1)

        def allgather(src_ap, dst_ap, pool):
            dummy = pool.tile([1, 1], src_ap.dtype, tag="agdummy")
            nc.gpsimd.dma_start(dummy[:], src_ap[0:1, 0:1])
            nc.gpsimd.collective_compute(
                "AllGather", mybir.AluOpType.bypass,
                replica_groups=[list(range(NC))],
                ins=[src_ap[:]], outs=[dst_ap[:]])

        # ---------------- edge phases ----------------
        def edge_phase(layer):
            EW = ewidth(layer)           # 136 / 11
            FW = 128 if layer < 3 else 10
            tab = table if layer < 3 else table3
            tw = TW if layer < 3 else TW3
            tdt = BF16 if layer < 3 else FP32
            off = 0
            with tc.tile_pool(name=f"ed{layer}", bufs=2) as pool, \
                 tc.tile_pool(name=f"eb{layer}", bufs=1) as bpool, \
                 tc.tile_pool(name=f"eb2{layer}", bufs=2) as b2pool, \
                 tc.tile_pool(name=f"eps{layer}", bufs=2, space="PSUM") as pps, \
                 tc.tile_pool(name=f"evs{layer}", bufs=2, space="PSUM") as vps, \
                 tc.tile_pool(name=f"efl{layer}", bufs=1, space="PSUM") as fps:
                for gi, ws in enumerate(groups):
                    cols = int(P[gi].sum()) // 128
                    G = pool.tile([128, cols, tw], tdt, tag="G")
                    ds = bpool.tile([128, cols], FP32, tag="ds")
                    df = bpool.tile([1, cols * 128], BF16, tag="df")
                    nc.sync.dma_start(ds[:], dstS[:, off // 128: off // 128 + cols])
                    nc.sync.dma_start(df[:], dstF[:, off: off + cols * 128])
                    co = 0
                    for s in range(NSUP):
                        pcols = int(P[gi, s]) // 128
                        if pcols == 0:
                            continue
                        nidx = int(P[gi, s])
                        it = pool.tile([128, nidx // 16], I16, tag="it")
                        nc.sync.dma_start(
                            it[:], idxT[:, (off + co * 128) // 16:(off + co * 128 + nidx) // 16])
                        for q0 in range(0, nidx, GCH):
                            q1 = min(q0 + GCH, nidx)
                            nc.gpsimd.dma_gather(
                                G[:, co + q0 // 128: co + q1 // 128, :],
                                tab[s * SUPR:(s + 1) * SUPR, :],
                                it[:, q0 // 16: q1 // 16], q1 - q0, q1 - q0, tw)
                        co += pcols
                    # one-hot builds for the whole group
                    NOVB = bool(os.environ.get('KERNEL_NOVB'))
                    dfr = bpool.tile([128, cols * 128], BF16, tag="dfr")
                    if not NOVB:
                        nc.gpsimd.partition_broadcast(dfr[:], df[:])
                    B = b2pool.tile([128, cols, 128], BF16, tag="B")
                    nc.vector.tensor_tensor(
                        B[:],
                        iota_row[:].rearrange("p d -> p () d").broadcast_to([128, cols, 128]),
                        ds[:].rearrange("p c -> p c ()").broadcast_to([128, cols, 128]),
                        mybir.AluOpType.is_equal)
                    Bt = b2pool.tile([128, cols, 128], BF16, tag="Bt")
                    if not NOVB:
                        nc.vector.tensor_scalar(
                            Bt[:], dfr[:].rearrange("p (c e) -> p c e", e=128),
                            iota_col[:], None, mybir.AluOpType.is_equal)
                    # v per edge via per-column broadcast matmuls
                    NH = 8 if layer < 3 else 1
                    vp = vps.tile([128, cols, NH], FP32, tag="vp")
                    aldt = ald if layer < 3 else ald3
                    for c, ents in enumerate([] if NOVB else pieces[gi]):
                        if len(ents) == 1:
                            wi = ents[0][0]
                            nc.tensor.matmul(
                                vp[:, c, :], Bt[:, c, :], aldt[:, ws[wi], :],
                                start=True, stop=True)
                        else:
                            for ei, (wi, plo, phi, st, sp) in enumerate(ents):
                                bm = pool.tile([128, 128], BF16, tag="btm", name="btm")
                                nc.vector.memset(bm[:], 0.0)
                                nc.vector.tensor_copy(bm[:, plo:phi], Bt[:, c, plo:phi])
                                nc.tensor.matmul(
                                    vp[:, c, :], bm[:], aldt[:, ws[wi], :],
                                    start=(ei == 0), stop=(ei == len(ents) - 1))
                    # u, t, p
                    if layer < 3:
                        u = G[:, :, 128:144].bitcast(FP32)      # [128, cols, 8]
                    else:
                        u = G[:, :, 10:11]
                    t = pool.tile([128, cols, NH], FP32, tag="t")
                    if NOVB:
                        nc.vector.tensor_copy(t[:], u)
                    else:
                        nc.vector.tensor_tensor(t[:], u, vp[:], mybir.AluOpType.add)
                    lr = pool.tile([128, cols, NH], FP32, tag="lr")
                    nc.vector.tensor_scalar(lr[:], t[:], NEG, None, mybir.AluOpType.mult)
                    nc.vector.tensor_tensor(lr[:], t[:], lr[:], mybir.AluOpType.max)
                    p = pool.tile([128, cols, NH], FP32, tag="p")
                    nc.scalar.activation(p[:], lr[:], mybir.ActivationFunctionType.Exp)
                    # rhs = [msg | p]
                    rhs = pool.tile([128, cols, EW], BF16, tag="rhs")
                    if layer < 3:
                        nc.vector.tensor_tensor(
                            rhs[:, :, 0:128].rearrange("p c (h f) -> p c h f", f=16),
                            G[:, :, 0:128].rearrange("p c (h f) -> p c h f", f=16),
                            p[:].rearrange("p c h -> p c h ()").broadcast_to([128, cols, 8, 16]),
                            mybir.AluOpType.mult)
                        nc.scalar.activation(rhs[:, :, 128:136], p[:],
                                             mybir.ActivationFunctionType.Copy)
                    else:
                        nc.vector.tensor_tensor(
                            rhs[:, :, 0:10],
                            G[:, :, 0:10],
                            p[:].broadcast_to([128, cols, 10]),
                            mybir.AluOpType.mult)
                        nc.scalar.activation(rhs[:, :, 10:11], p[:],
                                             mybir.ActivationFunctionType.Copy)
                    # segment reduce into per-window psum
                    aggs = {}
                    for c, ents in enumerate(pieces[gi]):
                        for (wi, plo, phi, st, sp) in ents:
                            if wi not in aggs:
                                aggs[wi] = pps.tile([128, EW], FP32, tag=f"agg{wi}",
                                                    name=f"agg{wi}")
                            if len(ents) == 1:
                                lhs = B[:, c, :]
                            else:
                                mk = pool.tile([128, 1], FP32, tag="mk", name="mk")
                                if plo == 0:
                                    nc.vector.tensor_scalar(mk[:], iota_col[:], float(phi),
                                                            None, mybir.AluOpType.is_lt)
                                else:
                                    nc.vector.tensor_scalar(mk[:], iota_col[:], float(plo),
                                                            None, mybir.AluOpType.is_ge)
                                bm2 = pool.tile([128, 128], BF16, tag="bm2", name="bm2")
                                nc.vector.tensor_scalar(bm2[:], B[:, c, :], mk[:],
                                                        None, mybir.AluOpType.mult)
                                lhs = bm2[:]
                            nc.tensor.matmul(aggs[wi][:], lhs,
                                             rhs[:, c, :], start=st, stop=sp)
                    # flush each window of this group
                    for wi, w in enumerate(ws):
                        ag = aggs[wi]
                        den = pool.tile([128, NH], FP32, tag="den")
                        nc.vector.reciprocal(den[:], ag[:, FW:EW])
                        z = pool.tile([128, FW], FP32, tag="z")
                        if layer < 3:
                            nc.vector.tensor_tensor(
                                z[:].rearrange("p (h f) -> p h f", f=16),
                                ag[:, 0:128].rearrange("p (h f) -> p h f", f=16),
                                den[:].rearrange("p h -> p h ()").broadcast_to([128, 8, 16]),
                                mybir.AluOpType.mult)
                        else:
                            nc.vector.tensor_tensor(
                                z[:], ag[:, 0:10],
                                den[:].broadcast_to([128, 10]),
                                mybir.AluOpType.mult)
                        bt = (b1t, b2t, b3t)[layer - 1]
                        nc.vector.tensor_tensor(z[:], z[:], bt[:, 0:FW], mybir.AluOpType.add)
                        if layer < 3:
                            # y = ELU(z); yT; table_next = yT.T @ Wnext + c
                            mn = pool.tile([128, FW], FP32, tag="mn")
                            nc.vector.tensor_scalar(mn[:], z[:], 0.0, None, mybir.AluOpType.min)
                            ex = pool.tile([128, FW], FP32, tag="ex")
                            nc.scalar.activation(ex[:], mn[:], mybir.ActivationFunctionType.Exp)
                            y = pool.tile([128, FW], FP32, tag="y")
                            nc.vector.tensor_scalar(y[:], z[:], 0.0, None, mybir.AluOpType.max)
                            nc.vector.tensor_tensor(y[:], y[:], ex[:], mybir.AluOpType.add)
                            nc.vector.tensor_scalar(y[:], y[:], 1.0, None, mybir.AluOpType.subtract)
                            trp = fps.tile([128, 128], FP32, tag="trp")
                            nc.tensor.transpose(trp[:], y[:], ident[:])
                            yT = pool.tile([128, 128], FP32, tag="yT")
                            nc.scalar.activation(yT[:], trp[:], mybir.ActivationFunctionType.Copy)
                            wnt = w2t if layer == 1 else w3t
                            cnt = c2t if layer == 1 else c3t
                            wdt = 144 if layer == 1 else 12
                            tp = fps.tile([128, wdt], FP32, tag="tp")
                            nc.tensor.matmul(tp[:], ones1[:], cnt[:], start=True, stop=False)
                            nc.tensor.matmul(tp[:], yT[:], wnt[:], start=False, stop=True)
                            table_row_epilogue(pool, tp, w, layer + 1)
                        else:
                            # log_softmax over 10 classes
                            m = pool.tile([128, 1], FP32, tag="m")
                            nc.vector.reduce_max(m[:], z[:], axis=mybir.AxisListType.X)
                            tt = pool.tile([128, 10], FP32, tag="tt")
                            nc.vector.tensor_scalar(tt[:], z[:], m[:], None,
                                                    mybir.AluOpType.subtract)
                            et = pool.tile([128, 10], FP32, tag="et")
                            ssum = pool.tile([128, 1], FP32, tag="ssum")
                            nc.scalar.activation(et[:], tt[:],
                                                 mybir.ActivationFunctionType.Exp,
                                                 accum_out=ssum[:])
                            ls = pool.tile([128, 1], FP32, tag="ls")
                            nc.scalar.activation(ls[:], ssum[:],
                                                 mybir.ActivationFunctionType.Ln)
                            ot = pool.tile([128, 10], FP32, tag="ot")
                            nc.vector.tensor_scalar(ot[:], tt[:], ls[:], None,
                                                    mybir.AluOpType.subtract)
                            rows = min(128, NSH - w * 128)
                            if rows > 0:
                                nc.sync.dma_start(
                                    out_ext[w * 128: w * 128 + rows, :], ot[0:rows, :])
                    off += cols * 128

        with tc.tile_pool(name="ag", bufs=1) as agp:
            allgather(agin.ap(), table.ap(), agp)
        edge_phase(1)
        with tc.tile_pool(name="ag2", bufs=1) as agp:
            allgather(agin.ap(), table.ap(), agp)
        edge_phase(2)
        with tc.tile_pool(name="ag3", bufs=1) as agp:
            allgather(agin3.ap(), table3.ap(), agp)
        edge_phase(3)

    nc.compile()
    return nc


def _host_inputs(inputs, struct):
    x = np.asarray(inputs['x'], np.float32)
    s1 = np.asarray(inputs['g1'], np.float32) / np.sqrt(1.0 + BN_EPS)
    s2 = np.asarray(inputs['g2'], np.float32) / np.sqrt(1.0 + BN_EPS)
    be1 = np.asarray(inputs['be1'], np.float32)
    be2 = np.asarray(inputs['be2'], np.float32)
    A1 = np.concatenate([_block_diag_att(np.asarray(inputs['a1_src'])),
                         _block_diag_att(np.asarray(inputs['a1_dst']))], 1)  # [128,16]
    A2 = np.concatenate([_block_diag_att(np.asarray(inputs['a2_src'])),
                         _block_diag_att(np.asarray(inputs['a2_dst']))], 1)
    A3 = np.concatenate([np.asarray(inputs['a3_src']).T,
                         np.asarray(inputs['a3_dst']).T], 1)                 # [10,2]
    W1 = np.asarray(inputs['W1'], np.float32)
    W2 = np.asarray(inputs['W2'], np.float32)
    W3 = np.asarray(inputs['W3'], np.float32)
    W1ext = np.concatenate([W1, W1 @ A1], 1)              # [500,144]
    W2ext = np.concatenate([W2, W2 @ A2], 1)              # [128,144]
    W3ext = np.concatenate([W3, W3 @ A3], 1)              # [128,12]
    W2eff = s1[:, None] * W2ext
    W3eff = s2[:, None] * W3ext
    c2 = (be1 @ W2ext).reshape(1, 144).astype(np.float32)
    c3 = (be2 @ W3ext).reshape(1, 12).astype(np.float32)
    W1p = np.zeros((512, 144), np.float32)
    W1p[:F_IN] = W1ext
    b1 = np.asarray(inputs['b1'], np.float32)
    b2 = np.asarray(inputs['b2'], np.float32)
    b3 = np.asarray(inputs['b3'], np.float32)

    maps = []
    for c in range(NC):
        xs = np.zeros((NSHP, 512), np.float32)
        xs[:NSH, :F_IN] = x[c * NSH:(c + 1) * NSH]
        maps.append({
            "xT": np.ascontiguousarray(xs.T.reshape(4, 128, NSHP)),
            "W1e": np.ascontiguousarray(W1p.reshape(4, 128, 144)),
            "W2e": W2eff, "W3e": W3eff, "c2r": c2, "c3r": c3,
            "b1r": np.tile(b1, (128, 1)), "b2r": np.tile(b2, (128, 1)),
            "b3r": np.tile(b3, (128, 1)),
            "idx": struct['idx_w'][c], "dsts": struct['dst_st'][c],
            "dstf": struct['dst_fl'][c],
        })
    return maps


def kernel(**inputs):
    ei = np.asarray(inputs['edge_index'])
    key = 'prog'
    if key not in _CACHE:
        struct = _preprocess(ei.astype(np.int64))
        nc = _build(struct)
        _CACHE[key] = (struct, nc)
    struct, nc = _CACHE[key]
    in_maps = _host_inputs(inputs, struct)
    t0 = time.time()
    res = run_bass_kernel_spmd(nc, in_maps, list(range(NC)),
                               trace=bool(os.environ.get('KERNEL_TRACE')))
    LAST['wall_s'] = time.time() - t0
    LAST['exec_time_ns'] = res.exec_time_ns
    LAST['res'] = res
    out = np.concatenate([res.results[c]["out"] for c in range(NC)], 0)
    return out.astype(np.float32)


if __name__ == "__main__":
    import reference
    inp = reference.setup_inputs()
    inp = {k: np.asarray(v) for k, v in inp.items()}
    out = kernel(**inp)
    print("kernel out", out.shape, out[:2, :3])
